# revision 1
# baseline (speedup 1.0000x reference)
"""Multi-head attention (B=2, S=2048, D=1024, H=16, DH=64) on 8 TRN2 cores.

Sharding: core c handles batch b = c//4 and head group g = c%4 (4 heads).
Per core, for its (b, g):
    VhT/KhT/QhT = per-head projections in transposed layout [e, s],
    Vh = PE-transposed back to [j, e] with a ones column appended (vhe),
    S^T = Kh @ Qh^T per head (scores transposed, keys j on partitions),
    P^T = exp(S^T / sqrt(dk))  (no max subtraction; fp32 range is ample),
    acc = Vh_ext^T @ P^T  (row 64 = softmax denominators via the ones col),
    outT = acc[0:64] * (1/l) broadcast  (PE ones-outer-product broadcast),
    PT_partial = sum_c Wf[c,:] outT[c,:]  -> partial final projection [D, S].
Host: out[b] = (sum_g PT_partial).T + bf.

Schedule: V and K stream first (full S), then Q streams in two i-halves;
attention + the final projection for each i-half overlap the later streams.

Data path is bf16: streams, weights, P (exp output), V tiles, and the
final output partials are all bf16 (halves DMA traffic; matmul rate on
TRN2 is the same 1 row/cycle for bf16 as float32r). PSUM accumulation
stays fp32.
"""

import sys

sys.path.insert(0, "/opt/trn_rl_repo")

from contextlib import ExitStack

import ml_dtypes
import numpy as np

import concourse.mybir as mybir
import concourse.tile as tile
from concourse import bacc
from concourse.bass_utils import run_bass_kernel_spmd

B, S, D, H, DH = 2, 2048, 1024, 16, 64
NCORES = 8
GPB = 4  # head-group cores per batch
HPG = H // GPB  # heads per group (4)
CW = HPG * DH  # concat width per core (256)
NPAIR = HPG // 2  # head pairs per group (2)
DCH = D // 128  # d chunks (8)
JCH = S // 128  # key chunks (16)
IB = 1024  # i-block width for attention
NIB = S // IB  # 2
F32 = mybir.dt.float32
BF16 = mybir.dt.bfloat16
FP8 = mybir.dt.float8e4
AF = mybir.ActivationFunctionType
INV_SQRT_DK = 1.0 / np.sqrt(DH)
BFNP = ml_dtypes.bfloat16

_CACHE = {}


def _build():
    nc = bacc.Bacc("TRN2", target_bir_lowering=False, debug=False, num_devices=NCORES)

    qt_d = nc.dram_tensor("qt", [D, S], BF16, kind="ExternalInput").ap()
    kt_d = nc.dram_tensor("kt", [D, S], BF16, kind="ExternalInput").ap()
    vt_d = nc.dram_tensor("vt", [D, S], BF16, kind="ExternalInput").ap()
    wq_d = nc.dram_tensor("wq", [D, CW], BF16, kind="ExternalInput").ap()
    wk_d = nc.dram_tensor("wk", [D, CW], BF16, kind="ExternalInput").ap()
    wv_d = nc.dram_tensor("wv", [D, CW], BF16, kind="ExternalInput").ap()
    wf_d = nc.dram_tensor("wf", [CW, D], BF16, kind="ExternalInput").ap()
    bq_d = nc.dram_tensor("bq", [CW], F32, kind="ExternalInput").ap()
    bk_d = nc.dram_tensor("bk", [CW], F32, kind="ExternalInput").ap()
    bv_d = nc.dram_tensor("bv", [1, CW], BF16, kind="ExternalInput").ap()
    ones_d = nc.dram_tensor("ones32", [128, 2 * JCH, 1], BF16, kind="ExternalInput").ap()
    onesr_d = nc.dram_tensor("ones_row", [1, 128], BF16, kind="ExternalInput").ap()
    pt_d = nc.dram_tensor("pt", [D, S], BF16, kind="ExternalOutput").ap()

    with (
        tile.TileContext(nc) as tc,
        nc.allow_low_precision(reason="bf16 data path is intentional"),
        ExitStack() as ctx,
    ):
        const = ctx.enter_context(tc.tile_pool(name="const", bufs=1))
        persist = ctx.enter_context(tc.tile_pool(name="persist", bufs=1))

        wq_sb = const.tile([128, DCH * CW], BF16, tag="wq")
        wk_sb = const.tile([128, DCH * CW], BF16, tag="wk")
        wv_sb = const.tile([128, DCH * CW], BF16, tag="wv")
        wf_sb = const.tile([128, 2 * D], BF16, tag="wf")
        bq_sb = const.tile([128, NPAIR], F32, tag="bq")
        bk_sb = const.tile([128, NPAIR], F32, tag="bk")
        bv_sb = const.tile([1, CW], BF16, tag="bv")
        ones128 = const.tile([1, 128], BF16, tag="ones")
        ones32 = const.tile([128, 2 * JCH, 1], BF16, tag="ones32")

        def load_w(w_sb, w_dram):
            nc.sync.dma_start(
                out=w_sb[:].rearrange("p (c e) -> p c e", c=DCH),
                in_=w_dram.rearrange("(c p) e -> p c e", p=128),
            )

        def load_b(b_sb, b_dram):
            nc.sync.dma_start(out=b_sb[:], in_=b_dram.rearrange("(r p) -> p r", p=128))

        qhT = [persist.tile([128, S], BF16, tag=f"qhT{r}", name=f"qhT{r}") for r in range(NPAIR)]
        outT = [persist.tile([128, S], BF16, tag=f"outT{r}", name=f"outT{r}") for r in range(NPAIR)]
        khT = [persist.tile([128, S], BF16, tag=f"khT{r}", name=f"khT{r}") for r in range(NPAIR)]
        vhe = [persist.tile([128, JCH * 130], BF16, tag=f"vhe{r}", name=f"vhe{r}") for r in range(NPAIR)]
        # fp8 hi/lo DoubleRow score operands for the LATE head-blocks only
        # (quantized in the background from the bf16 qhT/khT; head 0/1 keep
        # the bf16 score path so the startup chain is untouched). For head h:
        # mq[h] moving: parts 0:64 = fp8(qhT), 64:128 = fp8 residual, slabs
        # duplicated; stK[h] stationary: slab0 = fp8(khT), slab1 = residual,
        # partition halves duplicated. One DoubleRow matmul contracts
        # (qhi+qlo)(khi+klo) exactly at 0.5 cycles/row.
        mq = [persist.tile([128, 2, S], FP8, tag=f"mq{h}", name=f"mq{h}") for h in range(HPG)]
        stK = [persist.tile([128, 2, S], FP8, tag=f"stK{h}", name=f"stK{h}") for h in range(HPG)]

        # ============ V: project directly into natural [j, e] layout ============
        with tc.tile_pool(name="xt", bufs=8) as xt_pool:
            load_w(wv_sb, wv_d)
            nc.sync.dma_start(out=bv_sb[:], in_=bv_d)
            nc.sync.dma_start(out=ones128[:], in_=onesr_d)
            nc.sync.dma_start(out=ones32[:], in_=ones_d)
            xtv = []
            for d in range(DCH):
                t = xt_pool.tile([128, S], BF16, tag="xt", name="xt_v")
                nc.sync.dma_start(out=t[:], in_=vt_d[128 * d : 128 * (d + 1), :])
                xtv.append(t)
            with tc.tile_pool(name="ps_vh", bufs=8, space="PSUM") as ps_vh_pool, nc.named_scope("vproj"):
                ps_vh = [
                    ps_vh_pool.tile([128, 512], F32, tag="vh", name=f"ps_vh{jb}")
                    for jb in range(JCH // 2)
                ]
                for r in range(NPAIR):
                    nc.vector.tensor_copy(
                        vhe[r][:].rearrange("p (c w) -> p c w", w=65)[:, :, 64:65],
                        ones32[:],
                    )
                for jh in range(2):
                    for jb in range(JCH // 2):
                        j = 2 * jb + jh
                        reg = ps_vh[jb][:, 256 * jh : 256 * (jh + 1)]
                        for d in range(DCH):
                            nc.tensor.matmul(
                                reg,
                                xtv[d][:, 128 * j : 128 * (j + 1)],
                                wv_sb[:, CW * d : CW * (d + 1)],
                                start=(d == 0),
                                stop=False,
                            )
                        nc.tensor.matmul(
                            reg, ones128[:], bv_sb[:], start=False, stop=True
                        )
                        for r in range(NPAIR):
                            dst = vhe[r][:, 130 * j : 130 * j + 130]
                            nc.vector.tensor_copy(
                                dst.rearrange("p (b e) -> p b e", e=65)[:, :, 0:64],
                                reg[:, 128 * r : 128 * (r + 1)]
                                .rearrange("p (b e) -> p b e", e=64),
                            )

            load_w(wq_sb, wq_d)
            load_b(bq_sb, bq_d)
            load_w(wk_sb, wk_d)
            load_b(bk_sb, bk_d)

        # ========= attention-era pools open here =========
        if True:
            with (
                tc.tile_pool(name="qx", bufs=8) as qx_pool,
                tc.tile_pool(name="kx", bufs=10) as kx_pool,
                tc.tile_pool(name="pexp", bufs=6) as pexp_pool,
                tc.tile_pool(name="bc", bufs=2) as bc_pool,
                tc.tile_pool(name="rc", bufs=2) as rc_pool,
                tc.tile_pool(name="fo", bufs=4) as fo_pool,
                tc.tile_pool(name="hl8q", bufs=2) as hl8q_pool,
                tc.tile_pool(name="hl8k", bufs=2) as hl8k_pool,
                tc.tile_pool(name="ps_sc", bufs=2, space="PSUM") as ps_sc,
                tc.tile_pool(name="ps_acc", bufs=2, space="PSUM") as ps_acc,
            ):
                def emit_quantQ(ib_, r):
                    """Background fp8 hi/lo split of qhT[r] cols ib_ into
                    mq[2r], mq[2r+1] (DVE quantize + HWDGE dup; both idle
                    when this runs)."""
                    isl_ = slice(IB * ib_, IB * (ib_ + 1))
                    hl = hl8q_pool.tile([128, 2, IB], FP8, tag="hl8q", name="hl8q")
                    nc.vector.tensor_copy(hl[:, 0, :], qhT[r][:, isl_])
                    nc.vector.tensor_tensor(
                        hl[:, 1, :], qhT[r][:, isl_], hl[:, 0, :],
                        mybir.AluOpType.subtract,
                    )
                    for q in range(2):
                        h = 2 * r + q
                        ph = slice(64 * q, 64 * (q + 1))
                        nc.sync.dma_start(
                            out=mq[h][0:64, :, isl_],
                            in_=hl[ph, 0:1, :].broadcast_to([64, 2, IB]),
                        )
                        nc.sync.dma_start(
                            out=mq[h][64:128, :, isl_],
                            in_=hl[ph, 1:2, :].broadcast_to([64, 2, IB]),
                        )

                def emit_quantK(r):
                    """Background fp8 hi/lo split of the full khT[r] into
                    stK[2r], stK[2r+1] (gpsimd quantize, idle mid-run)."""
                    hl = hl8k_pool.tile([128, 2, S], FP8, tag="hl8k", name="hl8k")
                    nc.gpsimd.tensor_copy(hl[:, 0, :], khT[r][:])
                    nc.gpsimd.tensor_tensor(
                        hl[:, 1, :], khT[r][:], hl[:, 0, :],
                        mybir.AluOpType.subtract,
                    )
                    for q in range(2):
                        h = 2 * r + q
                        ph = slice(64 * q, 64 * (q + 1))
                        nc.sync.dma_start(out=stK[h][0:64, :, :], in_=hl[ph, :, :])
                        nc.sync.dma_start(out=stK[h][64:128, :, :], in_=hl[ph, :, :])

                def emit_scores_fp8(h, s_ps, jc, ib_):
                    k_st = stK[h][:, :, 128 * jc : 128 * (jc + 1)]
                    for k in range(IB // 512):
                        c0 = IB * ib_ + 512 * k
                        nc.tensor.matmul(
                            s_ps[:, 512 * k : 512 * (k + 1)],
                            k_st,
                            mq[h][:, :, c0 : c0 + 512],
                            start=True,
                            stop=True,
                            perf_mode=mybir.MatmulPerfMode.DoubleRow,
                        )

                def emit_q_dmas(ib_):
                    isl_ = slice(IB * ib_, IB * (ib_ + 1))
                    qx = []
                    for d in range(DCH):
                        t = qx_pool.tile([128, IB], BF16, tag="qx", name="qx")
                        nc.sync.dma_start(out=t[:], in_=qt_d[128 * d : 128 * (d + 1), isl_])
                        qx.append(t)
                    return qx

                def make_qproj_closures(ib_, r, qx):
                    isl_ = slice(IB * ib_, IB * (ib_ + 1))
                    state = {}

                    def step(d):
                        if d == 0:
                            state["ps"] = ps_acc.tile([128, IB], F32, tag="acc", name="ps_qd")
                        ps_q = state["ps"]
                        w_st = wq_sb[:, CW * d + 128 * r : CW * d + 128 * (r + 1)]
                        for k in range(IB // 512):
                            nc.tensor.matmul(
                                ps_q[:, 512 * k : 512 * (k + 1)],
                                w_st,
                                qx[d][:, 512 * k : 512 * (k + 1)],
                                start=(d == 0),
                                stop=(d == DCH - 1),
                            )

                    def bias():
                        nc.vector.tensor_scalar_add(
                            qhT[r][:, isl_], state["ps"][:], bq_sb[:, r : r + 1]
                        )

                    return [
                        (lambda d=d: step(d)) for d in range(DCH)
                    ] + [bias, lambda: emit_quantQ(ib_, r)]

                def emit_qproj_pair(ib_, r, qx):
                    with nc.named_scope(f"qproj{ib_}r{r}"):
                        isl_ = slice(IB * ib_, IB * (ib_ + 1))
                        ps_q = ps_sc.tile([128, IB], F32, tag="sc", name="ps_q")
                        for d in range(DCH):
                            w_st = wq_sb[:, CW * d + 128 * r : CW * d + 128 * (r + 1)]
                            for k in range(IB // 512):
                                nc.tensor.matmul(
                                    ps_q[:, 512 * k : 512 * (k + 1)],
                                    w_st,
                                    qx[d][:, 512 * k : 512 * (k + 1)],
                                    start=(d == 0),
                                    stop=(d == DCH - 1),
                                )
                        nc.vector.tensor_scalar_add(
                            qhT[r][:, isl_], ps_q[:], bq_sb[:, r : r + 1]
                        )

                # Q0 stream + projection; fp8 quantize runs in background
                qx0 = emit_q_dmas(0)
                emit_qproj_pair(0, 0, qx0)
                emit_qproj_pair(0, 1, qx0)
                emit_quantQ(0, 0)
                emit_quantQ(0, 1)


                def emit_kproj_pair(sblk, r, kx):
                    ps_kb = ps_sc.tile([128, 512], F32, tag="sc", name="ps_kb")
                    for d in range(DCH):
                        w_st = wk_sb[:, CW * d + 128 * r : CW * d + 128 * (r + 1)]
                        nc.tensor.matmul(
                            ps_kb[:],
                            w_st,
                            kx[d][:],
                            start=(d == 0),
                            stop=(d == DCH - 1),
                        )
                    nc.vector.tensor_scalar_add(
                        khT[r][:, 512 * sblk : 512 * (sblk + 1)],
                        ps_kb[:],
                        bk_sb[:, r : r + 1],
                    )

                def emit_final_tile(ib_, f, i4, pool=None, copy_eng=None):
                    i0 = IB * ib_ + 512 * i4
                    pf = (pool or ps_sc).tile(
                        [128, 512], F32, tag="sc" if pool is None else "acc", name="pf"
                    )
                    for cc in range(2):
                        nc.tensor.matmul(
                            pf[:],
                            wf_sb[:, D * cc + 128 * f : D * cc + 128 * (f + 1)],
                            outT[cc][:, i0 : i0 + 512],
                            start=(cc == 0),
                            stop=(cc == 1),
                        )
                    fo = fo_pool.tile([128, 512], BF16, tag="fo", name="fo")
                    if copy_eng == "act":
                        nc.scalar.copy(fo[:], pf[:])
                    else:
                        nc.vector.tensor_copy(fo[:], pf[:])
                    nc.sync.dma_start(
                        out=pt_d[128 * f : 128 * (f + 1), i0 : i0 + 512],
                        in_=fo[:],
                    )

                def emit_norm(pend):
                    acc_, r_, qs_, isl_ = pend
                    with nc.named_scope("norm"):
                        rc = rc_pool.tile([1, IB], BF16, tag="rc", name="rc")
                        nc.vector.reciprocal(rc[:], acc_[64:65, :])
                        bc_ps = ps_sc.tile([128, IB], F32, tag="sc", name="bc_ps")
                        for k in range(IB // 512):
                            nc.tensor.matmul(
                                bc_ps[0:64, 512 * k : 512 * (k + 1)],
                                ones128[0:1, 0:64],
                                rc[:, 512 * k : 512 * (k + 1)],
                                start=True,
                                stop=True,
                            )
                        bc_sb = bc_pool.tile([64, IB], F32, tag="bc", name="bc_sb")
                        nc.vector.tensor_copy(bc_sb[:], bc_ps[0:64, :])
                        nc.vector.tensor_tensor(
                            outT[r_][qs_, isl_],
                            acc_[0:64, :],
                            bc_sb[:],
                            mybir.AluOpType.mult,
                        )

                pending_norm_box = [None]

                def emit_attention(ib_, deferred, heads=range(HPG), flush_norm=True, fp8=()):
                    isl_ = slice(IB * ib_, IB * (ib_ + 1))
                    pending_norm = pending_norm_box[0]
                    for h in heads:
                        with nc.named_scope(f"attn{ib_}h{h}"):
                            r, q = h // 2, h % 2
                            qs = slice(64 * q, 64 * (q + 1))
                            acc = ps_acc.tile([65, IB], F32, tag="acc", name="acc")
                            prev_pv = None
                            for jc in range(JCH):
                                s_ps = ps_sc.tile([128, IB], F32, tag="sc", name="s_ps")
                                if h in fp8:
                                    emit_scores_fp8(h, s_ps, jc, ib_)
                                else:
                                    k_st = khT[r][qs, 128 * jc : 128 * (jc + 1)]
                                    for k in range(IB // 512):
                                        nc.tensor.matmul(
                                            s_ps[:, 512 * k : 512 * (k + 1)],
                                            k_st,
                                            qhT[r][qs, IB * ib_ + 512 * k : IB * ib_ + 512 * (k + 1)],
                                            start=True,
                                            stop=True,
                                        )
                                if prev_pv is not None:
                                    v_st_p, pexp_p, jc_p = prev_pv
                                    for k in range(IB // 512):
                                        nc.tensor.matmul(
                                            acc[:, 512 * k : 512 * (k + 1)],
                                            v_st_p,
                                            pexp_p[:, 512 * k : 512 * (k + 1)],
                                            start=(jc_p == 0),
                                            stop=False,
                                        )
                                pexp = pexp_pool.tile([128, IB], BF16, tag="pexp", name="pexp")
                                nc.scalar.activation(pexp[:], s_ps[:], AF.Exp, scale=INV_SQRT_DK)
                                prev_pv = (
                                    vhe[r][:, 130 * jc + 65 * q : 130 * jc + 65 * (q + 1)],
                                    pexp,
                                    jc,
                                )
                                if jc == 1 and pending_norm is not None:
                                    emit_norm(pending_norm)
                                    pending_norm = None
                                if jc in (3, 5, 7, 9, 11, 13) and deferred:
                                    deferred.pop(0)()
                            v_st_p, pexp_p, jc_p = prev_pv
                            for k in range(IB // 512):
                                nc.tensor.matmul(
                                    acc[:, 512 * k : 512 * (k + 1)],
                                    v_st_p,
                                    pexp_p[:, 512 * k : 512 * (k + 1)],
                                    start=False,
                                    stop=True,
                                )
                            pending_norm = (acc, r, qs, isl_)
                    while deferred:
                        deferred.pop(0)()
                    if flush_norm:
                        emit_norm(pending_norm)
                        pending_norm = None
                    pending_norm_box[0] = pending_norm

                def emit_k_dmas(sblk):
                    kx = []
                    for d in range(DCH):
                        t = kx_pool.tile([128, 512], BF16, tag="kx", name="kx")
                        nc.sync.dma_start(
                            out=t[:],
                            in_=kt_d[128 * d : 128 * (d + 1), 512 * sblk : 512 * (sblk + 1)],
                        )
                        kx.append(t)
                    return kx

                NSB = S // 512
                with nc.named_scope("chase"):
                    acc0 = ps_acc.tile([65, IB], F32, tag="acc", name="acc0")
                    prev_pv = None
                    kx_next = emit_k_dmas(0)
                    emit_kproj_pair(0, 0, kx_next)
                    emit_kproj_pair(0, 1, kx_next)
                    for sblk in range(NSB):
                        if sblk + 1 < NSB:
                            kx_next = emit_k_dmas(sblk + 1)
                        for jc in range(4 * sblk, 4 * sblk + 4):
                            s_ps = ps_sc.tile([128, IB], F32, tag="sc", name="s_ps")
                            for k in range(IB // 512):
                                nc.tensor.matmul(
                                    s_ps[:, 512 * k : 512 * (k + 1)],
                                    khT[0][0:64, 128 * jc : 128 * (jc + 1)],
                                    qhT[0][0:64, 512 * k : 512 * (k + 1)],
                                    start=True,
                                    stop=True,
                                )
                            if prev_pv is not None:
                                v_st_p, pexp_p, jc_p = prev_pv
                                for k in range(IB // 512):
                                    nc.tensor.matmul(
                                        acc0[:, 512 * k : 512 * (k + 1)],
                                        v_st_p,
                                        pexp_p[:, 512 * k : 512 * (k + 1)],
                                        start=(jc_p == 0),
                                        stop=False,
                                    )
                            pexp = pexp_pool.tile([128, IB], BF16, tag="pexp", name="pexp")
                            nc.scalar.activation(pexp[:], s_ps[:], AF.Exp, scale=INV_SQRT_DK)
                            prev_pv = (vhe[0][:, 130 * jc : 130 * jc + 65], pexp, jc)
                        if sblk + 1 < NSB:
                            emit_kproj_pair(sblk + 1, 0, kx_next)
                            emit_kproj_pair(sblk + 1, 1, kx_next)
                    v_st_p, pexp_p, jc_p = prev_pv
                    for k in range(IB // 512):
                        nc.tensor.matmul(
                            acc0[:, 512 * k : 512 * (k + 1)],
                            v_st_p,
                            pexp_p[:, 512 * k : 512 * (k + 1)],
                            start=False,
                            stop=True,
                        )
                    pending_norm_box[0] = (acc0, 0, slice(0, 64), slice(0, IB))

                # khT complete: quantize the fp8 stationaries in background
                # (r=1 first — heads 2,3 run soonest on the fp8 path)
                emit_quantK(1)
                emit_quantK(0)

                nc.sync.dma_start(
                    out=wf_sb[:].rearrange("p (c f) -> p c f", c=2),
                    in_=wf_d.rearrange("(c p) f -> p c f", p=128),
                )
                qx1 = emit_q_dmas(1)
                emit_attention(0, [], heads=[1, 2], flush_norm=False, fp8=(2,))
                emit_attention(0, make_qproj_closures(1, 0, qx1), heads=[3], fp8=(3,))
                final0 = [
                    (lambda f=f, i4=i4: emit_final_tile(0, f, i4))
                    for f in range(D // 128)
                    for i4 in range(IB // 512)
                ]
                emit_attention(
                    1, make_qproj_closures(1, 1, qx1), heads=[0, 1],
                    flush_norm=False, fp8=(0, 1),
                )
                emit_attention(1, final0, heads=[2, 3], flush_norm=False, fp8=(2, 3))
                acc_, r_, qs_, isl_ = pending_norm_box[0]
                pending_norm_box[0] = None
                with nc.named_scope("final1"):
                    for k in range(IB // 512):
                        i0 = IB + 512 * k
                        rc = rc_pool.tile([1, 512], BF16, tag="rc", name="rc")
                        nc.vector.reciprocal(rc[:], acc_[64:65, 512 * k : 512 * (k + 1)])
                        bc_ps = ps_sc.tile([128, 512], F32, tag="sc", name="bc_ps")
                        nc.tensor.matmul(
                            bc_ps[0:64, :], ones128[0:1, 0:64], rc[:], start=True, stop=True
                        )
                        bc_sb = bc_pool.tile([64, 512], F32, tag="bc", name="bc_sb")
                        nc.vector.tensor_copy(bc_sb[:], bc_ps[0:64, :])
                        nc.vector.tensor_tensor(
                            outT[r_][qs_, i0 : i0 + 512],
                            acc_[0:64, 512 * k : 512 * (k + 1)],
                            bc_sb[:],
                            mybir.AluOpType.mult,
                        )
                        for n, f in enumerate(range(D // 128)):
                            emit_final_tile(
                                1, f, k,
                                pool=(ps_acc if n % 2 else None),
                                copy_eng=("act" if n % 2 else None),
                            )

    nc.compile()
    return nc


def _get_nc():
    if "nc" not in _CACHE:
        _CACHE["nc"] = _build()
    return _CACHE["nc"]


def _bf(x):
    return np.ascontiguousarray(np.asarray(x, dtype=np.float32)).astype(BFNP)


def kernel(Q, K, V, Wq, bq, Wk, bk, Wv, bv, Wf, bf):
    Q, K, V = np.asarray(Q), np.asarray(K), np.asarray(V)
    Wq, Wk, Wv, Wf = (np.asarray(a) for a in (Wq, Wk, Wv, Wf))
    bq, bk, bv, bf = (np.asarray(a) for a in (bq, bk, bv, bf))

    nc = _get_nc()

    qt = [_bf(Q[b].T) for b in range(B)]
    kt = [_bf(K[b].T) for b in range(B)]
    vt = [_bf(V[b].T) for b in range(B)]
    wq_g = [_bf(Wq[HPG * g : HPG * (g + 1)].transpose(1, 0, 2).reshape(D, CW)) for g in range(GPB)]
    wk_g = [_bf(Wk[HPG * g : HPG * (g + 1)].transpose(1, 0, 2).reshape(D, CW)) for g in range(GPB)]
    wv_g = [_bf(Wv[HPG * g : HPG * (g + 1)].transpose(1, 0, 2).reshape(D, CW)) for g in range(GPB)]
    wf_g = [_bf(Wf[CW * g : CW * (g + 1), :]) for g in range(GPB)]
    bq_g = [np.ascontiguousarray(bq[HPG * g : HPG * (g + 1)].reshape(CW), np.float32) for g in range(GPB)]
    bk_g = [np.ascontiguousarray(bk[HPG * g : HPG * (g + 1)].reshape(CW), np.float32) for g in range(GPB)]
    bv_g = [_bf(bv[HPG * g : HPG * (g + 1)].reshape(1, CW)) for g in range(GPB)]

    ones_col = np.ones((128, 2 * JCH, 1), BFNP)
    ones_row = np.ones((1, 128), BFNP)
    in_maps = []
    for c in range(NCORES):
        b, g = c // GPB, c % GPB
        in_maps.append(
            {
                "qt": qt[b], "kt": kt[b], "vt": vt[b],
                "wq": wq_g[g], "wk": wk_g[g], "wv": wv_g[g], "wf": wf_g[g],
                "bq": bq_g[g], "bk": bk_g[g], "bv": bv_g[g],
                "ones32": ones_col, "ones_row": ones_row,
            }
        )

    res = run_bass_kernel_spmd(nc, in_maps, list(range(NCORES)))

    out = np.empty((B, S, D), np.float32)
    bf32 = bf.astype(np.float32)
    for b in range(B):
        acc = res.results[GPB * b]["pt"].astype(np.float32)
        for g in range(1, GPB):
            acc = acc + res.results[GPB * b + g]["pt"].astype(np.float32)
        out[b] = acc.T + bf32
    return out



# revision 19
# speedup vs baseline: 1.1325x; 1.1325x over previous
"""Multi-head attention (B=2, S=2048, D=1024, H=16, DH=64) on 8 TRN2 cores.

Sharding: core c handles batch b = c//4 and head group g = c%4 (4 heads).
Per core, for its (b, g):
    QhT/KhT = per-head projections in transposed layout [dh, s] (pairs r),
    Vh = projected directly into [j, e] layout with a ones column (vhe),
    S^T = Kh @ Qh^T per head (scores transposed, keys j on partitions),
    P^T = exp(S^T / sqrt(dk))  (no max subtraction; fp32 range is ample),
    acc[i, e] = sum_j P[j,i] V[j,e]   <- P is the matmul STATIONARY operand,
        V (65 cols incl. the ones col) is the moving operand, so each
        128-key chunk costs only 65 PE rows. Col 64 = softmax denominator.
    norm: out[i, e] = acc[i, 0:64] * (1/acc[i, 64])  (per-partition scalar),
    transpose via PE back to [e, i] (outT) for the final projection,
    PT_partial = Wf^T outT -> partial final projection [D, S].
Host: out[b] = (sum_g PT_partial).T + bf.

exp runs on three engines: ACT (exact table exp) plus a tunable fraction
on Pool/DVE using a Schraudolph-style bf16 bit-trick (y = s*alpha + beta,
truncate to int16, bitcast to bf16), rel RMS err ~1.8% on those tiles.

Schedule: W/Q/K0 stream first; chase era runs h0's full pipeline plus
h1's scores+exp (h1's PV deferred until h0's PSUM accumulators free),
interleaved with per-sblk K/V projection. h2/h3 + ib1 use fp8 DoubleRow
scores (hi/lo split, exact) and are ACT-bound with exp offload.
"""

import sys

sys.path.insert(0, "/opt/trn_rl_repo")

from contextlib import ExitStack

import ml_dtypes
import numpy as np

import concourse.mybir as mybir
import concourse.tile as tile
from concourse import bacc
from concourse.bass_utils import run_bass_kernel_spmd

B, S, D, H, DH = 2, 2048, 1024, 16, 64
NCORES = 8
GPB = 4  # head-group cores per batch
HPG = H // GPB  # heads per group (4)
CW = HPG * DH  # concat width per core (256)
NPAIR = HPG // 2  # head pairs per group (2)
DCH = D // 128  # d chunks (8)
JCH = S // 128  # key chunks (16)
IB = 1024  # i-block width for attention
NIB = S // IB  # 2
NSB = S // 512  # key sblks (4)
F32 = mybir.dt.float32
BF16 = mybir.dt.bfloat16
I16 = mybir.dt.int16
FP8 = mybir.dt.float8e4
AF = mybir.ActivationFunctionType
ALU = mybir.AluOpType
INV_SQRT_DK = 1.0 / np.sqrt(DH)
BFNP = ml_dtypes.bfloat16

# Schraudolph bf16 exp: i16 = trunc(s*EXP_A + EXP_B); bitcast i16 -> bf16.
EXP_A = float(128.0 * np.log2(np.e) * INV_SQRT_DK)
EXP_B = float(127 * 128 - 7.5 + 0.5)

# exp engine split for the ACT-bound blocks (h2,h3,ib1): jc slots on DVE
# (Pool/GPSIMD cannot access PSUM, so only DVE can read scores directly)
DVE_JCS = ()

_CACHE = {}


def _build():
    nc = bacc.Bacc("TRN2", target_bir_lowering=False, debug=False, num_devices=NCORES)

    qt_d = nc.dram_tensor("qt", [D, S], BF16, kind="ExternalInput").ap()
    kt_d = nc.dram_tensor("kt", [D, S], BF16, kind="ExternalInput").ap()
    vt_d = nc.dram_tensor("vt", [D, S], BF16, kind="ExternalInput").ap()
    wq_d = nc.dram_tensor("wq", [D, CW], BF16, kind="ExternalInput").ap()
    wk_d = nc.dram_tensor("wk", [D, CW], BF16, kind="ExternalInput").ap()
    wv_d = nc.dram_tensor("wv", [D, CW], BF16, kind="ExternalInput").ap()
    wf_d = nc.dram_tensor("wf", [CW, D], BF16, kind="ExternalInput").ap()
    bq_d = nc.dram_tensor("bq", [CW], F32, kind="ExternalInput").ap()
    bk_d = nc.dram_tensor("bk", [CW], F32, kind="ExternalInput").ap()
    bv_d = nc.dram_tensor("bv", [1, CW], BF16, kind="ExternalInput").ap()
    ones_d = nc.dram_tensor("ones32", [128, 2 * JCH, 1], BF16, kind="ExternalInput").ap()
    onesr_d = nc.dram_tensor("ones_row", [1, 128], BF16, kind="ExternalInput").ap()
    ident_d = nc.dram_tensor("ident", [128, 128], BF16, kind="ExternalInput").ap()
    pt_d = nc.dram_tensor("pt", [D, S], BF16, kind="ExternalOutput").ap()
    dbg_d = nc.dram_tensor("dbg", [NPAIR, 128, S], BF16, kind="ExternalOutput").ap()
    dbg2_d = nc.dram_tensor("dbg2", [128, 8, 65], F32, kind="ExternalOutput").ap()
    dbg3_d = nc.dram_tensor("dbg3", [128, 8, 64], BF16, kind="ExternalOutput").ap()

    with (
        tile.TileContext(nc) as tc,
        nc.allow_low_precision(reason="bf16/fp8 data path is intentional"),
        ExitStack() as ctx,
    ):
        const = ctx.enter_context(tc.tile_pool(name="const", bufs=1))
        persist = ctx.enter_context(tc.tile_pool(name="persist", bufs=1))

        wq_sb = const.tile([128, DCH * CW], BF16, tag="wq")
        wk_sb = const.tile([128, DCH * CW], BF16, tag="wk")
        wv_sb = const.tile([128, DCH * CW], BF16, tag="wv")
        wf_sb = const.tile([128, 2 * D], BF16, tag="wf")
        bq_sb = const.tile([128, NPAIR], F32, tag="bq")
        bk_sb = const.tile([128, NPAIR], F32, tag="bk")
        bv_sb = const.tile([1, CW], BF16, tag="bv")
        ones128 = const.tile([1, 128], BF16, tag="ones")
        ones32 = const.tile([128, 2 * JCH, 1], BF16, tag="ones32")
        ident_sb = const.tile([128, 128], BF16, tag="ident")

        qhT = [persist.tile([128, S], BF16, tag=f"qhT{r}", name=f"qhT{r}") for r in range(NPAIR)]
        khT = [persist.tile([128, S], BF16, tag=f"khT{r}", name=f"khT{r}") for r in range(NPAIR)]
        outT = [persist.tile([128, S], BF16, tag=f"outT{r}", name=f"outT{r}") for r in range(NPAIR)]
        vhe = [persist.tile([128, JCH * 130], BF16, tag=f"vhe{r}", name=f"vhe{r}") for r in range(NPAIR)]
        mq = [persist.tile([128, 2, S], FP8, tag=f"mq{h}", name=f"mq{h}") for h in range(HPG)]
        stK = [persist.tile([128, 2, S], FP8, tag=f"stK{h}", name=f"stK{h}") for h in range(HPG)]

        def load_w(w_sb, w_dram):
            nc.sync.dma_start(
                out=w_sb[:].rearrange("p (c e) -> p c e", c=DCH),
                in_=w_dram.rearrange("(c p) e -> p c e", p=128),
            )

        def load_b(b_sb, b_dram):
            nc.sync.dma_start(out=b_sb[:], in_=b_dram.rearrange("(r p) -> p r", p=128))

        with (
            tc.tile_pool(name="qx", bufs=8) as qx_pool,
            tc.tile_pool(name="kx", bufs=2) as kx_pool,
            tc.tile_pool(name="vx", bufs=2) as vx_pool,
            tc.tile_pool(name="pexp", bufs=8) as pexp_pool,
            tc.tile_pool(name="pexh1", bufs=16) as pexh1_pool,
            tc.tile_pool(name="aexp", bufs=4) as aexp_pool,
            tc.tile_pool(name="nrm", bufs=2) as nrm_pool,
            tc.tile_pool(name="rc", bufs=2) as rc_pool,
            tc.tile_pool(name="fo", bufs=4) as fo_pool,
            tc.tile_pool(name="hl8q", bufs=2) as hl8q_pool,
            tc.tile_pool(name="hl8k", bufs=2) as hl8k_pool,
            tc.tile_pool(name="ps_sc", bufs=2, space="PSUM") as sc_ps,
            tc.tile_pool(name="ps_ax", bufs=2, space="PSUM") as ax_ps,
        ):
            # ---------------- DMA kickoff (SP program order = stream order) --
            load_w(wq_sb, wq_d)
            load_w(wk_sb, wk_d)
            load_b(bq_sb, bq_d)
            load_b(bk_sb, bk_d)
            nc.sync.dma_start(out=ones128[:], in_=onesr_d)
            nc.sync.dma_start(out=ones32[:], in_=ones_d)
            nc.sync.dma_start(out=ident_sb[:], in_=ident_d)
            nc.sync.dma_start(out=bv_sb[:], in_=bv_d)

            def emit_q_dmas(ib_):
                isl_ = slice(IB * ib_, IB * (ib_ + 1))
                qx = []
                for d in range(DCH):
                    t = qx_pool.tile([128, IB], BF16, tag="qx", name="qx")
                    nc.sync.dma_start(out=t[:], in_=qt_d[128 * d : 128 * (d + 1), isl_])
                    qx.append(t)
                return qx

            def emit_kx(sblk):
                t = kx_pool.tile([128, DCH, 512], BF16, tag="kx", name="kx")
                nc.sync.dma_start(
                    out=t[:],
                    in_=kt_d.rearrange("(c p) s -> p c s", p=128)[
                        :, :, 512 * sblk : 512 * (sblk + 1)
                    ],
                )
                return t

            def emit_vx(sblk):
                t = vx_pool.tile([128, DCH, 512], BF16, tag="vx", name="vx")
                nc.sync.dma_start(
                    out=t[:],
                    in_=vt_d.rearrange("(c p) s -> p c s", p=128)[
                        :, :, 512 * sblk : 512 * (sblk + 1)
                    ],
                )
                return t

            qx0 = emit_q_dmas(0)
            kx = [None] * NSB
            vx = [None] * NSB
            kx[0] = emit_kx(0)
            vx[0] = emit_vx(0)
            load_w(wv_sb, wv_d)
            for sblk in range(1, NSB):
                kx[sblk] = emit_kx(sblk)
                vx[sblk] = emit_vx(sblk)
            qx1 = emit_q_dmas(1)
            nc.sync.dma_start(
                out=wf_sb[:].rearrange("p (c f) -> p c f", c=2),
                in_=wf_d.rearrange("(c p) f -> p c f", p=128),
            )

            # vhe ones columns (col 64 of each 65-block)
            for r in range(NPAIR):
                nc.vector.tensor_copy(
                    vhe[r][:].rearrange("p (c w) -> p c w", w=65)[:, :, 64:65],
                    ones32[:],
                )

            # ---------------- building blocks -------------------------------
            def emit_kproj(sblk, r, kx_t):
                ps = ax_ps.tile([128, 512], F32, tag="chp", name="ps_kb")
                for d in range(DCH):
                    nc.tensor.matmul(
                        ps[:],
                        wk_sb[:, CW * d + 128 * r : CW * d + 128 * (r + 1)],
                        kx_t[:, d, :],
                        start=(d == 0),
                        stop=(d == DCH - 1),
                    )
                nc.vector.tensor_scalar_add(
                    khT[r][:, 512 * sblk : 512 * (sblk + 1)], ps[:], bk_sb[:, r : r + 1]
                )

            def emit_qproj(ib_, r, qx):
                isl_ = slice(IB * ib_, IB * (ib_ + 1))
                ps_q = sc_ps.tile([128, IB], F32, tag="sc", name="ps_q")
                for d in range(DCH):
                    w_st = wq_sb[:, CW * d + 128 * r : CW * d + 128 * (r + 1)]
                    for k in range(IB // 512):
                        nc.tensor.matmul(
                            ps_q[:, 512 * k : 512 * (k + 1)],
                            w_st,
                            qx[d][:, 512 * k : 512 * (k + 1)],
                            start=(d == 0),
                            stop=(d == DCH - 1),
                        )
                nc.vector.tensor_scalar_add(qhT[r][:, isl_], ps_q[:], bq_sb[:, r : r + 1])

            def emit_vhe_chunk(jc, vx_t):
                """Project V keys 128*jc..+128 into vhe[*] (both pairs)."""
                jloc = jc % 4
                reg = ax_ps.tile([128, 256], F32, tag="chp", name="vreg")
                for d in range(DCH):
                    nc.tensor.matmul(
                        reg[:],
                        vx_t[:, d, 128 * jloc : 128 * (jloc + 1)],
                        wv_sb[:, CW * d : CW * (d + 1)],
                        start=(d == 0),
                        stop=False,
                    )
                nc.tensor.matmul(reg[:], ones128[:], bv_sb[:], start=False, stop=True)
                for r in range(NPAIR):
                    dst = vhe[r][:, 130 * jc : 130 * jc + 130]
                    nc.vector.tensor_copy(
                        dst.rearrange("p (b e) -> p b e", e=65)[:, :, 0:64],
                        reg[:, 128 * r : 128 * (r + 1)].rearrange("p (b e) -> p b e", e=64),
                    )

            def emit_quantQ(ib_, r):
                """fp8 hi/lo split of qhT[r] cols of block ib_ -> mq[2r], mq[2r+1]."""
                isl_ = slice(IB * ib_, IB * (ib_ + 1))
                hl = hl8q_pool.tile([128, 2, IB], FP8, tag="hl8q", name="hl8q")
                nc.gpsimd.tensor_copy(hl[:, 0, :], qhT[r][:, isl_])
                nc.gpsimd.tensor_tensor(
                    hl[:, 1, :], qhT[r][:, isl_], hl[:, 0, :], ALU.subtract
                )
                return hl

            def emit_quantQ_dmas(hl, isl_, r):
                for q in range(2):
                    h = 2 * r + q
                    ph = slice(64 * q, 64 * (q + 1))
                    nc.sync.dma_start(
                        out=mq[h][0:64, :, isl_],
                        in_=hl[ph, 0:1, :].broadcast_to([64, 2, IB]),
                    )
                    nc.sync.dma_start(
                        out=mq[h][64:128, :, isl_],
                        in_=hl[ph, 1:2, :].broadcast_to([64, 2, IB]),
                    )

            def emit_quantK(r):
                """fp8 hi/lo split of the full khT[r] into stK[2r], stK[2r+1]."""
                hl = hl8k_pool.tile([128, 2, S], FP8, tag="hl8k", name="hl8k")
                nc.gpsimd.tensor_copy(hl[:, 0, :], khT[r][:])
                nc.gpsimd.tensor_tensor(hl[:, 1, :], khT[r][:], hl[:, 0, :], ALU.subtract)
                return hl

            def emit_quantK_dmas(hl, r):
                for q in range(2):
                    h = 2 * r + q
                    ph = slice(64 * q, 64 * (q + 1))
                    nc.sync.dma_start(out=stK[h][0:64, :, :], in_=hl[ph, :, :])
                    nc.sync.dma_start(out=stK[h][64:128, :, :], in_=hl[ph, :, :])

            def emit_scores(h, jc, ib_, fp8):
                s_ps = sc_ps.tile([128, IB], F32, tag="sc", name="s_ps")
                r, q = h // 2, h % 2
                if fp8:
                    k_st = stK[h][:, :, 128 * jc : 128 * (jc + 1)]
                    for k in range(IB // 512):
                        c0 = IB * ib_ + 512 * k
                        nc.tensor.matmul(
                            s_ps[:, 512 * k : 512 * (k + 1)],
                            k_st,
                            mq[h][:, :, c0 : c0 + 512],
                            start=True,
                            stop=True,
                            perf_mode=mybir.MatmulPerfMode.DoubleRow,
                        )
                else:
                    qs = slice(64 * q, 64 * (q + 1))
                    k_st = khT[r][qs, 128 * jc : 128 * (jc + 1)]
                    for k in range(IB // 512):
                        c0 = IB * ib_ + 512 * k
                        nc.tensor.matmul(
                            s_ps[:, 512 * k : 512 * (k + 1)],
                            k_st,
                            qhT[r][qs, c0 : c0 + 512],
                            start=True,
                            stop=True,
                        )
                return s_ps

            def emit_exp(s_ps, eng, pool=None):
                """exp(s/sqrt(dk)) -> [128, IB] bf16 stationary-capable AP."""
                if eng == "act":
                    t = (pool or pexp_pool).tile([128, IB], BF16, tag="pexp", name="pexp")
                    nc.scalar.activation(t[:], s_ps[:], AF.Exp, scale=INV_SQRT_DK)
                    return t[:]
                t = aexp_pool.tile([128, IB], I16, tag="aexp", name="aexp")
                nc.vector.tensor_scalar(t[:], s_ps[:], EXP_A, EXP_B, ALU.mult, ALU.add)
                return t[:].bitcast(BF16)

            def emit_pv(h, jc, pex, acc_lo, acc_hi):
                r, q = h // 2, h % 2
                vmov = vhe[r][:, 130 * jc + 65 * q : 130 * jc + 65 * (q + 1)]
                # one PSUM zero-region (bank) per acc tile: start only on the
                # first slice written, stop only on the last
                for ic in range(8):
                    tgt = (acc_lo if ic < 4 else acc_hi)[:, ic % 4, :]
                    nc.tensor.matmul(
                        tgt,
                        pex[:, 128 * ic : 128 * (ic + 1)],
                        vmov,
                        start=(jc == 0 and ic % 4 == 0),
                        stop=(jc == JCH - 1 and ic % 4 == 3),
                        skip_group_check=True,
                    )

            def emit_norm_tp(h, ib_, acc_lo, acc_hi):
                """normalize, transpose to [e, i], copy into outT."""
                r, q = h // 2, h % 2
                isl_ = slice(IB * ib_, IB * (ib_ + 1))
                qs = slice(64 * q, 64 * (q + 1))
                rc = rc_pool.tile([128, 8, 1], F32, tag="rc", name="rc")
                nc.vector.reciprocal(rc[:, 0:4, :], acc_lo[:, :, 64:65])
                nc.vector.reciprocal(rc[:, 4:8, :], acc_hi[:, :, 64:65])
                nrm = nrm_pool.tile([128, 8, 64], BF16, tag="nrm", name="nrm")
                for ic in range(8):
                    acc_t = acc_lo if ic < 4 else acc_hi
                    nc.vector.tensor_scalar_mul(
                        nrm[:, ic, :], acc_t[:, ic % 4, 0:64], rc[:, ic : ic + 1, 0]
                    )
                if h == 0 and ib_ == 0:
                    dacc = nrm_pool.tile([128, 8, 65], F32, tag="dacc", name="dacc")
                    nc.vector.tensor_copy(dacc[:, 0:4, :], acc_lo[:])
                    nc.vector.tensor_copy(dacc[:, 4:8, :], acc_hi[:])
                    nc.sync.dma_start(out=dbg2_d, in_=dacc[:])
                    nc.sync.dma_start(out=dbg3_d, in_=nrm[:])
                psT = ax_ps.tile([128, IB], BF16, tag="chp", name="psT")
                for ic in range(8):
                    nc.tensor.transpose(
                        psT[qs, 128 * ic : 128 * (ic + 1)], nrm[:, ic, :], ident_sb[:]
                    )
                nc.vector.tensor_copy(outT[r][qs, isl_], psT[qs, :])

            def emit_final_tile(ib_, f, i4, eng="dve"):
                i0 = IB * ib_ + 512 * i4
                pf = ax_ps.tile([128, 512], F32, tag="chp", name="pf")
                for cc in range(2):
                    nc.tensor.matmul(
                        pf[:],
                        wf_sb[:, D * cc + 128 * f : D * cc + 128 * (f + 1)],
                        outT[cc][:, i0 : i0 + 512],
                        start=(cc == 0),
                        stop=(cc == 1),
                    )
                fo = fo_pool.tile([128, 512], BF16, tag="fo", name="fo")
                if eng == "act":
                    nc.scalar.copy(fo[:], pf[:])
                else:
                    nc.vector.tensor_copy(fo[:], pf[:])
                nc.sync.dma_start(
                    out=pt_d[128 * f : 128 * (f + 1), i0 : i0 + 512], in_=fo[:]
                )

            # ---------------- pre-chase projections --------------------------
            with nc.named_scope("qproj0"):
                emit_qproj(0, 0, qx0)
                emit_qproj(0, 1, qx0)
            with nc.named_scope("kproj0"):
                emit_kproj(0, 0, kx[0])
                emit_kproj(0, 1, kx[0])
            hlq0 = emit_quantQ(0, 1)  # mq[2], mq[3] (DVE; dup DMAs deferred)

            # ---------------- chase era: h0 full + h1 scores/exp -------------
            # h1's pexp tiles are kept alive (pool depth) and PV'd in a burst
            # once h0's accumulators are normed and freed.
            acc_lo0 = None
            acc_hi0 = None
            h1_pex = []
            with nc.named_scope("chase"):
                pend0 = None
                for sblk in range(NSB):
                    for jloc in range(4):
                        jc = 4 * sblk + jloc
                        emit_vhe_chunk(jc, vx[sblk])
                        s0 = emit_scores(0, jc, 0, fp8=False)
                        p0 = emit_exp(s0, "act")
                        s1 = emit_scores(1, jc, 0, fp8=False)
                        p1 = emit_exp(s1, "act", pool=pexh1_pool)
                        h1_pex.append(p1)
                        if jc == 0:
                            acc_lo0 = ax_ps.tile([128, 4, 65], F32, tag="acc", name="acc_lo")
                            acc_hi0 = ax_ps.tile([128, 4, 65], F32, tag="acc", name="acc_hi")
                        emit_pv(0, jc, p0, acc_lo0, acc_hi0)
                    if sblk + 1 < NSB:
                        emit_kproj(sblk + 1, 0, kx[sblk + 1])
                        emit_kproj(sblk + 1, 1, kx[sblk + 1])
                    if sblk == 0:
                        emit_quantQ_dmas(hlq0, slice(0, IB), 1)

            with nc.named_scope("h0tail"):
                emit_norm_tp(0, 0, acc_lo0, acc_hi0)
            # khT complete: quantize fp8 stationaries (r=1 first, needed soonest)
            hlk1 = emit_quantK(1)
            emit_quantK_dmas(hlk1, 1)

            with nc.named_scope("h1burst"):
                acc_lo1 = ax_ps.tile([128, 4, 65], F32, tag="acc", name="acc_lo")
                acc_hi1 = ax_ps.tile([128, 4, 65], F32, tag="acc", name="acc_hi")
                for jc in range(JCH):
                    emit_pv(1, jc, h1_pex[jc], acc_lo1, acc_hi1)
                emit_norm_tp(1, 0, acc_lo1, acc_hi1)
            h1_pex = None

            hlk0 = emit_quantK(0)
            emit_quantK_dmas(hlk0, 0)

            # ---------------- steady blocks ----------------------------------
            def exp_engine(jc):
                if jc in DVE_JCS:
                    return "dve"
                return "act"

            def emit_block(h, ib_, deferred, pre_scores=()):
                """One ACT-bound head block with fp8 scores + exp offload.

                pre_scores: list of (h', jc, ib') score tiles already emitted
                for this block by the previous block (cross-block pipelining);
                returns the s_ps handles it pre-emits for the next block.
                """
                with nc.named_scope(f"blk{ib_}h{h}"):
                    sq = list(pre_scores)
                    acc_lo = acc_hi = None
                    pend = None
                    for jc in range(JCH):
                        if sq:
                            s_ps = sq.pop(0)
                        else:
                            s_ps = emit_scores(h, jc, ib_, fp8=True)
                        pex = emit_exp(s_ps, exp_engine(jc))
                        if jc == 0:
                            acc_lo = ax_ps.tile([128, 4, 65], F32, tag="acc", name="acc_lo")
                            acc_hi = ax_ps.tile([128, 4, 65], F32, tag="acc", name="acc_hi")
                        if pend is not None:
                            emit_pv(h, jc - 1, pend, acc_lo, acc_hi)
                        pend = pex
                        # next-block score pre-emission at the end of our run
                        if jc >= JCH - 2 and deferred:
                            pass
                        if jc in (3, 5, 7, 9, 11, 13) and deferred:
                            deferred.pop(0)()
                    while deferred:
                        deferred.pop(0)()
                    emit_pv(h, JCH - 1, pend, acc_lo, acc_hi)
                    emit_norm_tp(h, ib_, acc_lo, acc_hi)

            # h2, h3 of ib0; qproj-ib1 + quantQ-ib1 interleaved
            def defer_qproj(ib_, r):
                def f():
                    with nc.named_scope(f"qproj{ib_}r{r}"):
                        emit_qproj(ib_, r, qx1)

                return f

            def defer_quantQ(ib_, r):
                def f():
                    hl = emit_quantQ(ib_, r)
                    emit_quantQ_dmas(hl, slice(IB * ib_, IB * (ib_ + 1)), r)

                return f

            emit_block(2, 0, [defer_qproj(1, 0), defer_quantQ(1, 0)])
            emit_block(3, 0, [defer_qproj(1, 1), defer_quantQ(1, 1)])

            # ib1 blocks with ib0 finals interleaved
            finals0 = [
                (lambda f=f, i4=i4: emit_final_tile(0, f, i4))
                for f in range(D // 128)
                for i4 in range(IB // 512)
            ]
            emit_block(0, 1, finals0[0:6])
            emit_block(1, 1, finals0[6:12])
            emit_block(2, 1, finals0[12:16])
            emit_block(3, 1, [])

            # tail: ib1 finals
            with nc.named_scope("final1"):
                for f in range(D // 128):
                    for i4 in range(IB // 512):
                        emit_final_tile(1, f, i4, eng="act" if f % 2 else "dve")

            for r in range(NPAIR):
                nc.sync.dma_start(out=dbg_d[r], in_=outT[r][:])

    nc.compile()
    return nc


def _get_nc():
    if "nc" not in _CACHE:
        _CACHE["nc"] = _build()
    return _CACHE["nc"]


def _bf(x):
    return np.ascontiguousarray(np.asarray(x, dtype=np.float32)).astype(BFNP)


def kernel(Q, K, V, Wq, bq, Wk, bk, Wv, bv, Wf, bf):
    Q, K, V = np.asarray(Q), np.asarray(K), np.asarray(V)
    Wq, Wk, Wv, Wf = (np.asarray(a) for a in (Wq, Wk, Wv, Wf))
    bq, bk, bv, bf = (np.asarray(a) for a in (bq, bk, bv, bf))

    nc = _get_nc()

    qt = [_bf(Q[b].T) for b in range(B)]
    kt = [_bf(K[b].T) for b in range(B)]
    vt = [_bf(V[b].T) for b in range(B)]
    wq_g = [_bf(Wq[HPG * g : HPG * (g + 1)].transpose(1, 0, 2).reshape(D, CW)) for g in range(GPB)]
    wk_g = [_bf(Wk[HPG * g : HPG * (g + 1)].transpose(1, 0, 2).reshape(D, CW)) for g in range(GPB)]
    wv_g = [_bf(Wv[HPG * g : HPG * (g + 1)].transpose(1, 0, 2).reshape(D, CW)) for g in range(GPB)]
    wf_g = [_bf(Wf[CW * g : CW * (g + 1), :]) for g in range(GPB)]
    bq_g = [np.ascontiguousarray(bq[HPG * g : HPG * (g + 1)].reshape(CW), np.float32) for g in range(GPB)]
    bk_g = [np.ascontiguousarray(bk[HPG * g : HPG * (g + 1)].reshape(CW), np.float32) for g in range(GPB)]
    bv_g = [_bf(bv[HPG * g : HPG * (g + 1)].reshape(1, CW)) for g in range(GPB)]

    ones_col = np.ones((128, 2 * JCH, 1), BFNP)
    ones_row = np.ones((1, 128), BFNP)
    ident = np.eye(128, dtype=np.float32).astype(BFNP)
    in_maps = []
    for c in range(NCORES):
        b, g = c // GPB, c % GPB
        in_maps.append(
            {
                "qt": qt[b], "kt": kt[b], "vt": vt[b],
                "wq": wq_g[g], "wk": wk_g[g], "wv": wv_g[g], "wf": wf_g[g],
                "bq": bq_g[g], "bk": bk_g[g], "bv": bv_g[g],
                "ones32": ones_col, "ones_row": ones_row, "ident": ident,
            }
        )

    res = run_bass_kernel_spmd(nc, in_maps, list(range(NCORES)))

    out = np.empty((B, S, D), np.float32)
    bf32 = bf.astype(np.float32)
    for b in range(B):
        acc = res.results[GPB * b]["pt"].astype(np.float32)
        for g in range(1, GPB):
            acc = acc + res.results[GPB * b + g]["pt"].astype(np.float32)
        out[b] = acc.T + bf32
    return out


# revision 23
# speedup vs baseline: 1.1587x; 1.0232x over previous
"""Multi-head attention (B=2, S=2048, D=1024, H=16, DH=64) on 8 TRN2 cores.

Sharding: core c handles batch b = c//4 and head group g = c%4 (4 heads).
Per core, for its (b, g):
    QhT/KhT = per-head projections in transposed layout [dh, s] (pairs r),
    Vh = projected directly into [j, e] layout with a ones column (vhe),
    S^T = Kh @ Qh^T per head (scores transposed, keys j on partitions),
    P^T = exp(S^T / sqrt(dk))  (no max subtraction; fp32 range is ample),
    acc[i, e] = sum_j P[j,i] V[j,e]   <- P is the matmul STATIONARY operand,
        V (65 cols incl. the ones col) is the moving operand, so each
        128-key chunk costs only 65 PE rows. Col 64 = softmax denominator.
    norm: out[i, e] = acc[i, 0:64] * (1/acc[i, 64])  (per-partition scalar),
    transpose via PE back to [e, i] (outT) for the final projection,
    PT_partial = Wf^T outT -> partial final projection [D, S].
Host: out[b] = (sum_g PT_partial).T + bf.

exp runs on three engines: ACT (exact table exp) plus a tunable fraction
on Pool/DVE using a Schraudolph-style bf16 bit-trick (y = s*alpha + beta,
truncate to int16, bitcast to bf16), rel RMS err ~1.8% on those tiles.

Schedule: W/Q/K0 stream first; chase era runs h0's full pipeline plus
h1's scores+exp (h1's PV deferred until h0's PSUM accumulators free),
interleaved with per-sblk K/V projection. h2/h3 + ib1 use fp8 DoubleRow
scores (hi/lo split, exact) and are ACT-bound with exp offload.
"""

import sys

sys.path.insert(0, "/opt/trn_rl_repo")

from contextlib import ExitStack

import ml_dtypes
import numpy as np

import concourse.mybir as mybir
import concourse.tile as tile
from concourse import bacc
from concourse.bass_utils import run_bass_kernel_spmd

B, S, D, H, DH = 2, 2048, 1024, 16, 64
NCORES = 8
GPB = 4  # head-group cores per batch
HPG = H // GPB  # heads per group (4)
CW = HPG * DH  # concat width per core (256)
NPAIR = HPG // 2  # head pairs per group (2)
DCH = D // 128  # d chunks (8)
JCH = S // 128  # key chunks (16)
IB = 1024  # i-block width for attention
NIB = S // IB  # 2
NSB = S // 512  # key sblks (4)
F32 = mybir.dt.float32
BF16 = mybir.dt.bfloat16
I16 = mybir.dt.int16
FP8 = mybir.dt.float8e4
AF = mybir.ActivationFunctionType
ALU = mybir.AluOpType
INV_SQRT_DK = 1.0 / np.sqrt(DH)
BFNP = ml_dtypes.bfloat16

# Schraudolph bf16 exp: i16 = trunc(s*EXP_A + EXP_B); bitcast i16 -> bf16.
EXP_A = float(128.0 * np.log2(np.e) * INV_SQRT_DK)
EXP_B = float(127 * 128 - 7.5 + 0.5)

# exp engine split for the ACT-bound blocks (h2,h3,ib1): jc slots on DVE
# (Pool/GPSIMD cannot access PSUM, so only DVE can read scores directly)
DVE_JCS = (2, 7, 12)

_CACHE = {}


def _build():
    nc = bacc.Bacc("TRN2", target_bir_lowering=False, debug=False, num_devices=NCORES)

    qt_d = nc.dram_tensor("qt", [D, S], BF16, kind="ExternalInput").ap()
    kt_d = nc.dram_tensor("kt", [D, S], BF16, kind="ExternalInput").ap()
    vt_d = nc.dram_tensor("vt", [D, S], BF16, kind="ExternalInput").ap()
    wq_d = nc.dram_tensor("wq", [D, CW], BF16, kind="ExternalInput").ap()
    wk_d = nc.dram_tensor("wk", [D, CW], BF16, kind="ExternalInput").ap()
    wv_d = nc.dram_tensor("wv", [D, CW], BF16, kind="ExternalInput").ap()
    wf_d = nc.dram_tensor("wf", [CW, D], BF16, kind="ExternalInput").ap()
    bq_d = nc.dram_tensor("bq", [CW], F32, kind="ExternalInput").ap()
    bk_d = nc.dram_tensor("bk", [CW], F32, kind="ExternalInput").ap()
    bv_d = nc.dram_tensor("bv", [1, CW], BF16, kind="ExternalInput").ap()
    ones_d = nc.dram_tensor("ones32", [128, 2 * JCH, 1], BF16, kind="ExternalInput").ap()
    onesr_d = nc.dram_tensor("ones_row", [1, 128], BF16, kind="ExternalInput").ap()
    ident_d = nc.dram_tensor("ident", [128, 128], BF16, kind="ExternalInput").ap()
    pt_d = nc.dram_tensor("pt", [D, S], BF16, kind="ExternalOutput").ap()

    with (
        tile.TileContext(nc) as tc,
        nc.allow_low_precision(reason="bf16/fp8 data path is intentional"),
        ExitStack() as ctx,
    ):
        const = ctx.enter_context(tc.tile_pool(name="const", bufs=1))
        persist = ctx.enter_context(tc.tile_pool(name="persist", bufs=1))

        wq_sb = const.tile([128, DCH * CW], BF16, tag="wq")
        wk_sb = const.tile([128, DCH * CW], BF16, tag="wk")
        wv_sb = const.tile([128, DCH * CW], BF16, tag="wv")
        wf_sb = const.tile([128, 2 * D], BF16, tag="wf")
        bq_sb = const.tile([128, NPAIR], F32, tag="bq")
        bk_sb = const.tile([128, NPAIR], F32, tag="bk")
        bv_sb = const.tile([1, CW], BF16, tag="bv")
        ones128 = const.tile([1, 128], BF16, tag="ones")
        ones32 = const.tile([128, 2 * JCH, 1], BF16, tag="ones32")
        ident_sb = const.tile([128, 128], BF16, tag="ident")

        qhT = [persist.tile([128, S], BF16, tag=f"qhT{r}", name=f"qhT{r}") for r in range(NPAIR)]
        khT = [persist.tile([128, S], BF16, tag=f"khT{r}", name=f"khT{r}") for r in range(NPAIR)]
        outT = [persist.tile([128, S], BF16, tag=f"outT{r}", name=f"outT{r}") for r in range(NPAIR)]
        vhe = [persist.tile([128, JCH * 130], BF16, tag=f"vhe{r}", name=f"vhe{r}") for r in range(NPAIR)]
        mq = [persist.tile([128, 2, S], FP8, tag=f"mq{h}", name=f"mq{h}") for h in range(HPG)]
        stK = [persist.tile([128, 2, S], FP8, tag=f"stK{h}", name=f"stK{h}") for h in range(HPG)]

        def load_w(w_sb, w_dram):
            nc.sync.dma_start(
                out=w_sb[:].rearrange("p (c e) -> p c e", c=DCH),
                in_=w_dram.rearrange("(c p) e -> p c e", p=128),
            )

        def load_b(b_sb, b_dram):
            nc.sync.dma_start(out=b_sb[:], in_=b_dram.rearrange("(r p) -> p r", p=128))

        with (
            tc.tile_pool(name="qx", bufs=8) as qx_pool,
            tc.tile_pool(name="kx", bufs=2) as kx_pool,
            tc.tile_pool(name="vx", bufs=2) as vx_pool,
            tc.tile_pool(name="pexp", bufs=8) as pexp_pool,
            tc.tile_pool(name="pexh1", bufs=16) as pexh1_pool,
            tc.tile_pool(name="aexp", bufs=4) as aexp_pool,
            tc.tile_pool(name="nrm", bufs=2) as nrm_pool,
            tc.tile_pool(name="rc", bufs=2) as rc_pool,
            tc.tile_pool(name="fo", bufs=4) as fo_pool,
            tc.tile_pool(name="hl8q", bufs=2) as hl8q_pool,
            tc.tile_pool(name="hl8k", bufs=2) as hl8k_pool,
            tc.tile_pool(name="ps_sc", bufs=2, space="PSUM") as sc_ps,
            tc.tile_pool(name="ps_ax", bufs=2, space="PSUM") as ax_ps,
        ):
            # ---------------- DMA kickoff (SP program order = stream order) --
            load_w(wq_sb, wq_d)
            load_w(wk_sb, wk_d)
            load_b(bq_sb, bq_d)
            load_b(bk_sb, bk_d)
            nc.sync.dma_start(out=ones128[:], in_=onesr_d)
            nc.sync.dma_start(out=ones32[:], in_=ones_d)
            nc.sync.dma_start(out=ident_sb[:], in_=ident_d)
            nc.sync.dma_start(out=bv_sb[:], in_=bv_d)

            def emit_q_dmas(ib_):
                isl_ = slice(IB * ib_, IB * (ib_ + 1))
                qx = []
                for d in range(DCH):
                    t = qx_pool.tile([128, IB], BF16, tag="qx", name="qx")
                    nc.sync.dma_start(out=t[:], in_=qt_d[128 * d : 128 * (d + 1), isl_])
                    qx.append(t)
                return qx

            def emit_kx(sblk):
                t = kx_pool.tile([128, DCH, 512], BF16, tag="kx", name="kx")
                nc.sync.dma_start(
                    out=t[:],
                    in_=kt_d.rearrange("(c p) s -> p c s", p=128)[
                        :, :, 512 * sblk : 512 * (sblk + 1)
                    ],
                )
                return t

            def emit_vx(sblk):
                t = vx_pool.tile([128, DCH, 512], BF16, tag="vx", name="vx")
                nc.sync.dma_start(
                    out=t[:],
                    in_=vt_d.rearrange("(c p) s -> p c s", p=128)[
                        :, :, 512 * sblk : 512 * (sblk + 1)
                    ],
                )
                return t

            qx0 = emit_q_dmas(0)
            kx = [None] * NSB
            vx = [None] * NSB
            kx[0] = emit_kx(0)
            vx[0] = emit_vx(0)
            load_w(wv_sb, wv_d)
            for sblk in range(1, NSB):
                kx[sblk] = emit_kx(sblk)
                vx[sblk] = emit_vx(sblk)
            qx1 = emit_q_dmas(1)
            nc.sync.dma_start(
                out=wf_sb[:].rearrange("p (c f) -> p c f", c=2),
                in_=wf_d.rearrange("(c p) f -> p c f", p=128),
            )

            # vhe ones columns (col 64 of each 65-block)
            for r in range(NPAIR):
                nc.vector.tensor_copy(
                    vhe[r][:].rearrange("p (c w) -> p c w", w=65)[:, :, 64:65],
                    ones32[:],
                )

            # ---------------- building blocks -------------------------------
            def emit_kproj(sblk, r, kx_t):
                ps = ax_ps.tile([128, 512], F32, tag="chp", name="ps_kb")
                for d in range(DCH):
                    nc.tensor.matmul(
                        ps[:],
                        wk_sb[:, CW * d + 128 * r : CW * d + 128 * (r + 1)],
                        kx_t[:, d, :],
                        start=(d == 0),
                        stop=(d == DCH - 1),
                    )
                nc.vector.tensor_scalar_add(
                    khT[r][:, 512 * sblk : 512 * (sblk + 1)], ps[:], bk_sb[:, r : r + 1]
                )

            def emit_qproj(ib_, r, qx):
                isl_ = slice(IB * ib_, IB * (ib_ + 1))
                ps_q = sc_ps.tile([128, IB], F32, tag="sc", name="ps_q")
                for d in range(DCH):
                    w_st = wq_sb[:, CW * d + 128 * r : CW * d + 128 * (r + 1)]
                    for k in range(IB // 512):
                        nc.tensor.matmul(
                            ps_q[:, 512 * k : 512 * (k + 1)],
                            w_st,
                            qx[d][:, 512 * k : 512 * (k + 1)],
                            start=(d == 0),
                            stop=(d == DCH - 1),
                        )
                nc.vector.tensor_scalar_add(qhT[r][:, isl_], ps_q[:], bq_sb[:, r : r + 1])

            def emit_vhe_chunk(jc, vx_t):
                """Project V keys 128*jc..+128 into vhe[*] (both pairs)."""
                jloc = jc % 4
                reg = ax_ps.tile([128, 256], F32, tag="chp", name="vreg")
                for d in range(DCH):
                    nc.tensor.matmul(
                        reg[:],
                        vx_t[:, d, 128 * jloc : 128 * (jloc + 1)],
                        wv_sb[:, CW * d : CW * (d + 1)],
                        start=(d == 0),
                        stop=False,
                    )
                nc.tensor.matmul(reg[:], ones128[:], bv_sb[:], start=False, stop=True)
                for r in range(NPAIR):
                    dst = vhe[r][:, 130 * jc : 130 * jc + 130]
                    nc.vector.tensor_copy(
                        dst.rearrange("p (b e) -> p b e", e=65)[:, :, 0:64],
                        reg[:, 128 * r : 128 * (r + 1)].rearrange("p (b e) -> p b e", e=64),
                    )

            def emit_quantQ(ib_, r):
                """fp8 hi/lo split of qhT[r] cols of block ib_ -> mq[2r], mq[2r+1]."""
                isl_ = slice(IB * ib_, IB * (ib_ + 1))
                hl = hl8q_pool.tile([128, 2, IB], FP8, tag="hl8q", name="hl8q")
                nc.gpsimd.tensor_copy(hl[:, 0, :], qhT[r][:, isl_])
                nc.gpsimd.tensor_tensor(
                    hl[:, 1, :], qhT[r][:, isl_], hl[:, 0, :], ALU.subtract
                )
                return hl

            def emit_quantQ_dmas(hl, isl_, r):
                for q in range(2):
                    h = 2 * r + q
                    ph = slice(64 * q, 64 * (q + 1))
                    nc.sync.dma_start(
                        out=mq[h][0:64, :, isl_],
                        in_=hl[ph, 0:1, :].broadcast_to([64, 2, IB]),
                    )
                    nc.sync.dma_start(
                        out=mq[h][64:128, :, isl_],
                        in_=hl[ph, 1:2, :].broadcast_to([64, 2, IB]),
                    )

            def emit_quantK(r):
                """fp8 hi/lo split of the full khT[r] into stK[2r], stK[2r+1]."""
                hl = hl8k_pool.tile([128, 2, S], FP8, tag="hl8k", name="hl8k")
                nc.gpsimd.tensor_copy(hl[:, 0, :], khT[r][:])
                nc.gpsimd.tensor_tensor(hl[:, 1, :], khT[r][:], hl[:, 0, :], ALU.subtract)
                return hl

            def emit_quantK_dmas(hl, r):
                for q in range(2):
                    h = 2 * r + q
                    ph = slice(64 * q, 64 * (q + 1))
                    nc.sync.dma_start(out=stK[h][0:64, :, :], in_=hl[ph, :, :])
                    nc.sync.dma_start(out=stK[h][64:128, :, :], in_=hl[ph, :, :])

            def emit_scores(h, jc, ib_, fp8):
                s_ps = sc_ps.tile([128, IB], F32, tag="sc", name="s_ps")
                r, q = h // 2, h % 2
                if fp8:
                    k_st = stK[h][:, :, 128 * jc : 128 * (jc + 1)]
                    for k in range(IB // 512):
                        c0 = IB * ib_ + 512 * k
                        nc.tensor.matmul(
                            s_ps[:, 512 * k : 512 * (k + 1)],
                            k_st,
                            mq[h][:, :, c0 : c0 + 512],
                            start=True,
                            stop=True,
                            perf_mode=mybir.MatmulPerfMode.DoubleRow,
                        )
                else:
                    qs = slice(64 * q, 64 * (q + 1))
                    k_st = khT[r][qs, 128 * jc : 128 * (jc + 1)]
                    for k in range(IB // 512):
                        c0 = IB * ib_ + 512 * k
                        nc.tensor.matmul(
                            s_ps[:, 512 * k : 512 * (k + 1)],
                            k_st,
                            qhT[r][qs, c0 : c0 + 512],
                            start=True,
                            stop=True,
                        )
                return s_ps

            def emit_exp(s_ps, eng, pool=None):
                """exp(s/sqrt(dk)) -> [128, IB] bf16 stationary-capable AP."""
                if eng == "act":
                    t = (pool or pexp_pool).tile([128, IB], BF16, tag="pexp", name="pexp")
                    nc.scalar.activation(t[:], s_ps[:], AF.Exp, scale=INV_SQRT_DK)
                    return t[:]
                t = aexp_pool.tile([128, IB], I16, tag="aexp", name="aexp")
                nc.vector.tensor_scalar(t[:], s_ps[:], EXP_A, EXP_B, ALU.mult, ALU.add)
                return t[:].bitcast(BF16)

            def emit_pv(h, jc, pex, acc_lo, acc_hi):
                r, q = h // 2, h % 2
                vmov = vhe[r][:, 130 * jc + 65 * q : 130 * jc + 65 * (q + 1)]
                # one PSUM zero-region (bank) per acc tile: start only on the
                # first slice written, stop only on the last
                for ic in range(8):
                    tgt = (acc_lo if ic < 4 else acc_hi)[:, ic % 4, :]
                    nc.tensor.matmul(
                        tgt,
                        pex[:, 128 * ic : 128 * (ic + 1)],
                        vmov,
                        start=(jc == 0 and ic % 4 == 0),
                        stop=(jc == JCH - 1 and ic % 4 == 3),
                        skip_group_check=True,
                    )

            def emit_norm_tp(h, ib_, acc_lo, acc_hi):
                """normalize, transpose to [e, i], copy into outT."""
                r, q = h // 2, h % 2
                isl_ = slice(IB * ib_, IB * (ib_ + 1))
                qs = slice(64 * q, 64 * (q + 1))
                rc = rc_pool.tile([128, 8, 1], F32, tag="rc", name="rc")
                nc.vector.reciprocal(rc[:, 0:4, :], acc_lo[:, :, 64:65])
                nc.vector.reciprocal(rc[:, 4:8, :], acc_hi[:, :, 64:65])
                nrm = nrm_pool.tile([128, 8, 64], BF16, tag="nrm", name="nrm")
                for ic in range(8):
                    acc_t = acc_lo if ic < 4 else acc_hi
                    nc.vector.tensor_scalar_mul(
                        nrm[:, ic, :], acc_t[:, ic % 4, 0:64], rc[:, ic : ic + 1, 0]
                    )

                psT = ax_ps.tile([128, IB], BF16, tag="chp", name="psT")
                for ic in range(8):
                    nc.tensor.transpose(
                        psT[qs, 128 * ic : 128 * (ic + 1)], nrm[:, ic, :], ident_sb[:]
                    )
                nc.vector.tensor_copy(outT[r][qs, isl_], psT[qs, :])

            def emit_final_tile(ib_, f, i4, eng="dve"):
                i0 = IB * ib_ + 512 * i4
                pf = ax_ps.tile([128, 512], F32, tag="chp", name="pf")
                for cc in range(2):
                    nc.tensor.matmul(
                        pf[:],
                        wf_sb[:, D * cc + 128 * f : D * cc + 128 * (f + 1)],
                        outT[cc][:, i0 : i0 + 512],
                        start=(cc == 0),
                        stop=(cc == 1),
                    )
                fo = fo_pool.tile([128, 512], BF16, tag="fo", name="fo")
                if eng == "act":
                    nc.scalar.copy(fo[:], pf[:])
                else:
                    nc.vector.tensor_copy(fo[:], pf[:])
                nc.sync.dma_start(
                    out=pt_d[128 * f : 128 * (f + 1), i0 : i0 + 512], in_=fo[:]
                )

            # ---------------- pre-chase projections --------------------------
            with nc.named_scope("qproj0"):
                emit_qproj(0, 0, qx0)
                emit_qproj(0, 1, qx0)
            with nc.named_scope("kproj0"):
                emit_kproj(0, 0, kx[0])
                emit_kproj(0, 1, kx[0])
            hlq0 = emit_quantQ(0, 1)  # mq[2], mq[3] (DVE; dup DMAs deferred)

            # ---------------- chase era: h0 full + h1 scores/exp -------------
            # h1's pexp tiles are kept alive (pool depth) and PV'd in a burst
            # once h0's accumulators are normed and freed.
            acc_lo0 = None
            acc_hi0 = None
            h1_pex = []
            with nc.named_scope("chase"):
                pend0 = None
                for sblk in range(NSB):
                    for jloc in range(4):
                        jc = 4 * sblk + jloc
                        emit_vhe_chunk(jc, vx[sblk])
                        s0 = emit_scores(0, jc, 0, fp8=False)
                        p0 = emit_exp(s0, "act")
                        s1 = emit_scores(1, jc, 0, fp8=False)
                        p1 = emit_exp(s1, "act", pool=pexh1_pool)
                        h1_pex.append(p1)
                        if jc == 0:
                            acc_lo0 = ax_ps.tile([128, 4, 65], F32, tag="acc", name="acc_lo")
                            acc_hi0 = ax_ps.tile([128, 4, 65], F32, tag="acc", name="acc_hi")
                        emit_pv(0, jc, p0, acc_lo0, acc_hi0)
                    if sblk + 1 < NSB:
                        emit_kproj(sblk + 1, 0, kx[sblk + 1])
                        emit_kproj(sblk + 1, 1, kx[sblk + 1])
                    if sblk == 0:
                        emit_quantQ_dmas(hlq0, slice(0, IB), 1)

            with nc.named_scope("h0tail"):
                emit_norm_tp(0, 0, acc_lo0, acc_hi0)
            # khT complete: quantize fp8 stationaries (r=1 first, needed soonest)
            hlk1 = emit_quantK(1)
            emit_quantK_dmas(hlk1, 1)

            with nc.named_scope("h1burst"):
                acc_lo1 = ax_ps.tile([128, 4, 65], F32, tag="acc", name="acc_lo")
                acc_hi1 = ax_ps.tile([128, 4, 65], F32, tag="acc", name="acc_hi")
                for jc in range(JCH):
                    emit_pv(1, jc, h1_pex[jc], acc_lo1, acc_hi1)
                emit_norm_tp(1, 0, acc_lo1, acc_hi1)
            h1_pex = None

            hlk0 = emit_quantK(0)
            emit_quantK_dmas(hlk0, 0)

            # ---------------- steady blocks ----------------------------------
            def exp_engine(jc):
                if jc in DVE_JCS:
                    return "dve"
                return "act"

            def emit_block(h, ib_, deferred, pre_scores=()):
                """One ACT-bound head block with fp8 scores + exp offload.

                pre_scores: list of (h', jc, ib') score tiles already emitted
                for this block by the previous block (cross-block pipelining);
                returns the s_ps handles it pre-emits for the next block.
                """
                with nc.named_scope(f"blk{ib_}h{h}"):
                    sq = list(pre_scores)
                    acc_lo = acc_hi = None
                    pend = None
                    for jc in range(JCH):
                        if sq:
                            s_ps = sq.pop(0)
                        else:
                            s_ps = emit_scores(h, jc, ib_, fp8=True)
                        pex = emit_exp(s_ps, exp_engine(jc))
                        if jc == 0:
                            acc_lo = ax_ps.tile([128, 4, 65], F32, tag="acc", name="acc_lo")
                            acc_hi = ax_ps.tile([128, 4, 65], F32, tag="acc", name="acc_hi")
                        if pend is not None:
                            emit_pv(h, jc - 1, pend, acc_lo, acc_hi)
                        pend = pex
                        # next-block score pre-emission at the end of our run
                        if jc >= JCH - 2 and deferred:
                            pass
                        if jc in (3, 5, 7, 9, 11, 13) and deferred:
                            deferred.pop(0)()
                    while deferred:
                        deferred.pop(0)()
                    emit_pv(h, JCH - 1, pend, acc_lo, acc_hi)
                    emit_norm_tp(h, ib_, acc_lo, acc_hi)

            # h2, h3 of ib0; qproj-ib1 + quantQ-ib1 interleaved
            def defer_qproj(ib_, r):
                def f():
                    with nc.named_scope(f"qproj{ib_}r{r}"):
                        emit_qproj(ib_, r, qx1)

                return f

            def defer_quantQ(ib_, r):
                def f():
                    hl = emit_quantQ(ib_, r)
                    emit_quantQ_dmas(hl, slice(IB * ib_, IB * (ib_ + 1)), r)

                return f

            emit_block(2, 0, [defer_qproj(1, 0), defer_quantQ(1, 0)])
            emit_block(3, 0, [defer_qproj(1, 1), defer_quantQ(1, 1)])

            # ib1 blocks with ib0 finals interleaved
            finals0 = [
                (lambda f=f, i4=i4: emit_final_tile(0, f, i4))
                for f in range(D // 128)
                for i4 in range(IB // 512)
            ]
            emit_block(0, 1, finals0[0:6])
            emit_block(1, 1, finals0[6:12])
            emit_block(2, 1, finals0[12:16])
            emit_block(3, 1, [])

            # tail: ib1 finals
            with nc.named_scope("final1"):
                for f in range(D // 128):
                    for i4 in range(IB // 512):
                        emit_final_tile(1, f, i4, eng="act" if f % 2 else "dve")

    nc.compile()
    return nc


def _get_nc():
    if "nc" not in _CACHE:
        _CACHE["nc"] = _build()
    return _CACHE["nc"]


def _bf(x):
    return np.ascontiguousarray(np.asarray(x, dtype=np.float32)).astype(BFNP)


def kernel(Q, K, V, Wq, bq, Wk, bk, Wv, bv, Wf, bf):
    Q, K, V = np.asarray(Q), np.asarray(K), np.asarray(V)
    Wq, Wk, Wv, Wf = (np.asarray(a) for a in (Wq, Wk, Wv, Wf))
    bq, bk, bv, bf = (np.asarray(a) for a in (bq, bk, bv, bf))

    nc = _get_nc()

    qt = [_bf(Q[b].T) for b in range(B)]
    kt = [_bf(K[b].T) for b in range(B)]
    vt = [_bf(V[b].T) for b in range(B)]
    wq_g = [_bf(Wq[HPG * g : HPG * (g + 1)].transpose(1, 0, 2).reshape(D, CW)) for g in range(GPB)]
    wk_g = [_bf(Wk[HPG * g : HPG * (g + 1)].transpose(1, 0, 2).reshape(D, CW)) for g in range(GPB)]
    wv_g = [_bf(Wv[HPG * g : HPG * (g + 1)].transpose(1, 0, 2).reshape(D, CW)) for g in range(GPB)]
    wf_g = [_bf(Wf[CW * g : CW * (g + 1), :]) for g in range(GPB)]
    bq_g = [np.ascontiguousarray(bq[HPG * g : HPG * (g + 1)].reshape(CW), np.float32) for g in range(GPB)]
    bk_g = [np.ascontiguousarray(bk[HPG * g : HPG * (g + 1)].reshape(CW), np.float32) for g in range(GPB)]
    bv_g = [_bf(bv[HPG * g : HPG * (g + 1)].reshape(1, CW)) for g in range(GPB)]

    ones_col = np.ones((128, 2 * JCH, 1), BFNP)
    ones_row = np.ones((1, 128), BFNP)
    ident = np.eye(128, dtype=np.float32).astype(BFNP)
    in_maps = []
    for c in range(NCORES):
        b, g = c // GPB, c % GPB
        in_maps.append(
            {
                "qt": qt[b], "kt": kt[b], "vt": vt[b],
                "wq": wq_g[g], "wk": wk_g[g], "wv": wv_g[g], "wf": wf_g[g],
                "bq": bq_g[g], "bk": bk_g[g], "bv": bv_g[g],
                "ones32": ones_col, "ones_row": ones_row, "ident": ident,
            }
        )

    res = run_bass_kernel_spmd(nc, in_maps, list(range(NCORES)))

    out = np.empty((B, S, D), np.float32)
    bf32 = bf.astype(np.float32)
    for b in range(B):
        acc = res.results[GPB * b]["pt"].astype(np.float32)
        for g in range(1, GPB):
            acc = acc + res.results[GPB * b + g]["pt"].astype(np.float32)
        out[b] = acc.T + bf32
    return out
